# revision 38
# baseline (speedup 1.0000x reference)
"""Trainium2 Bass kernel for multi-head self-attention with Q=K=V=x@Wq.

Problem: x [4, 2048, 512] f32, Wq [512, 512] f32, HEAD=8 (head_dim=64).
  q = x @ Wq;  per (b, h): S = q_h q_h^T / 8; out = softmax(S) @ q_h.

Sharding (8 cores): core i -> batch b = i//2, head group g = i%2 (4 heads).
Each core gets x[b] (pre-transposed on host) and Wq[:, 256g:256g+256];
produces out[b, :, 256g:256g+256].  No cross-core communication.

On-core algorithm (v3):
  Host supplies x^T interleaved as [128, ib, dc, i] so no PE transposes of x
  are needed.  Startup (software-pipelined per 128-row block ib):
    - DMA x^T block; q_nat(ib) = x(ib) @ Wq via 4 PE matmuls (contract d);
      DVE cast into q_nat [128, 16, 4, 65] bf16 whose 65th column is 1.0
    - q_T via 2 PE transposes of q_nat(ib) (2 heads stacked per transpose:
      q_T [128, 2, 2048] bf16 holds head 2p on partitions 0-63, head 2p+1 on
      64-127; bf16 tolerates the partition-offset / tile_position, f32r
      would not)
  Main loop per (h=(p,sub), half=1024 queries), 16 key blocks jb:
    - S = q_T_jb^T q_T_half (2 bf16 matmuls N=512, partition base 64*sub)
    - eb = exp(0.125*S) via ScalarE ACT (PSUM->SBUF, bf16), no accumulator
    - ctx_T[d|Z, i] += q_nat[jb, h, 0:65]^T eb: the 65th (ones) stationary
      column makes PSUM row 64 accumulate Z_i = sum_a E[a, i] for free
    - S(jb)/exp(jb) emitted before ctx(jb-1) so PE overlaps ACT
  Tail per (h, half), interleaved into the NEXT (h, half)'s jb loop:
    - DVE copy cp [65,1024] -> SBUF; 8 PE transposes [65,128] -> [128,65]
      (dedicated PSUM bank pool - sharing the S-tile pool rotation couples
      the PE to the DVE normalize chain and trips the HAM util-throttle);
    - DVE reciprocal of col 64 -> rz; DVE tensor_scalar_mul; DMA out.
  HAM: the PE de-warms (50% clock, sticky) on any sustained util dip, so the
  warmup runs dep-free matmuls immediately, the startup is stagger-pipelined
  to stay dense, and dummy matmuls pad the final drain.
  No max-subtraction needed: diag(S)/8 ~ 8 dominates, exp stays in f32 range.
"""

import sys

sys.path.insert(0, "/opt/trn_rl_repo")

from contextlib import ExitStack

import numpy as np

import concourse.bass as bass
import concourse.tile as tile
from concourse import bacc, mybir
from concourse.masks import make_identity

B, S, D, HEAD = 4, 2048, 512, 8
HD = D // HEAD  # 64
EC = 256  # e-columns per core (4 heads)
F32 = mybir.dt.float32
F32R = mybir.dt.float32r
BF16 = mybir.dt.bfloat16
N_CORES = 8

_PROGRAM = None


def build_program():
    nc = bacc.Bacc(None, target_bir_lowering=False)
    # x^T, host-interleaved: xt[p, (ib, dc, i)] = x[ib*128 + i, dc*128 + p]
    xt_d = nc.dram_tensor("xt", [128, 16 * 4 * 128], F32R, kind="ExternalInput")
    wq_d = nc.dram_tensor("wq", [D, EC], F32R, kind="ExternalInput")
    out_d = nc.dram_tensor("out", [S, EC], F32, kind="ExternalOutput")

    xt_r = xt_d.rearrange("p (s e) -> s p e", s=8)  # [8, 128, 1024] (2ib dc i)
    wq_r = wq_d.rearrange("(dc p) e -> p dc e", p=128)  # [128, 4, 256]
    out_r = out_d.rearrange("(ib p) e -> ib p e", p=128)  # [16, 128, 256]

    with tile.TileContext(nc) as tc, ExitStack() as ctx:
        sb = ctx.enter_context(tc.tile_pool(name="sb", bufs=1))
        xb = ctx.enter_context(tc.tile_pool(name="xb", bufs=3))
        ep = ctx.enter_context(tc.tile_pool(name="ep", bufs=6))
        csbp = ctx.enter_context(tc.tile_pool(name="csbp", bufs=2))
        ob = ctx.enter_context(tc.tile_pool(name="ob", bufs=6))
        rzp = ctx.enter_context(tc.tile_pool(name="rzp", bufs=6))
        ps = ctx.enter_context(tc.tile_pool(name="ps", bufs=2, space="PSUM"))
        cps = ctx.enter_context(tc.tile_pool(name="cps", bufs=1, space="PSUM"))
        tpp = ctx.enter_context(tc.tile_pool(name="tpp", bufs=1, space="PSUM"))
        wfp = ctx.enter_context(tc.tile_pool(name="wfp", bufs=1, space="PSUM"))

        # Warmup / HAM ignition first, with no dependency beyond one memset:
        # the PE powers on clock-throttled and needs ~3.4us of wait-free
        # matmuls to reach full clock before the real work starts.
        wrm = sb.tile([128, 512], F32R)
        nc.vector.memset(wrm.bitcast(F32), 0.0)
        wfil = wfp.tile([65, 512], F32, tag="wf")

        def dummy_mm():
            nc.tensor.matmul(wfil, wrm[:, 0:65], wrm, start=True, stop=True)

        for i in range(9):
            dummy_mm()
        # Trigger the one-time ~2.7us exp table load NOW (ACT is idle).
        dume = sb.tile([128, 1], F32)
        nc.scalar.activation(
            dume, wrm[:, 0:1].bitcast(F32), mybir.ActivationFunctionType.Exp
        )

        ident = sb.tile([128, 128], F32)
        make_identity(nc, ident)
        ident_b = sb.tile([128, 128], BF16)
        nc.vector.tensor_copy(ident_b, ident)

        wq_sb = sb.tile([128, 4, EC], F32R)
        nc.sync.dma_start(out=wq_sb, in_=wq_r)

        # q_T pair-stacked bf16; q_nat bf16 with the all-ones 65th column.
        q_T = sb.tile([128, 2, S], BF16)  # [sub*64+e, p, i]
        q_nat = sb.tile([128, 16, 4, 65], BF16)  # [j_in_block, jb, h, e|1]
        nc.vector.memset(q_nat[:, :, :, 64:65], 1.0)
        # pair-contiguous copy of q (no ones column) for the q_T transposes:
        # matmul stationary APs allow only one free dim, which the 65-stride
        # layout of q_nat cannot provide for a [128, 2, 64] slice.
        q_pair = sb.tile([128, 16, 2, 128], BF16)  # [j, jb, p, sub*64+e]

        # ---- Startup: software-pipelined q_nat, q_T (2 blocks per step;
        # the doubled PE work per step keeps the stream naturally dense) ----
        for step in range(9):
            # dep-free filler cushion at the step top: the step's first PE op
            # may wait on the previous step's ScalarE/DVE copies
            dummy_mm()
            if step == 8:
                for _ in range(5):
                    dummy_mm()
            if step < 8:
                xt = xb.tile([128, 2, 4, 128], F32R, tag="xt")
                nc.sync.dma_start(out=xt, in_=xt_r[step])
                qn = ps.tile([128, 2, 4, 64], F32, tag="ps")
                for k in range(2):
                    for dc in range(4):
                        nc.tensor.matmul(
                            qn[:, k, :, :],
                            xt[:, k, dc, :],
                            wq_sb[:, dc, :],
                            start=(dc == 0),
                            stop=(dc == 3),
                        )
                nc.vector.tensor_copy(
                    q_nat[:, 2 * step : 2 * step + 2, :, 0:64], qn
                )
                nc.scalar.copy(q_pair[:, 2 * step : 2 * step + 2, :, :], qn)
            if 1 <= step:
                sb0 = step - 1
                qtp = ps.tile([128, 2, 2, 128], BF16, tag="ps")  # [., p, k, i]
                for p in range(2):
                    for k in range(2):
                        nc.tensor.transpose(
                            qtp[:, p, k, :], q_pair[:, 2 * sb0 + k, p, :], ident_b
                        )
                nc.vector.tensor_copy(
                    q_T[:, :, sb0 * 256 : (sb0 + 1) * 256], qtp
                )

        # bridge burst: keeps the PE busy while the tail of the startup DVE
        # chain (final q_T copies) completes, so the main loop enters warm
        for i in range(20):
            dummy_mm()

        # ---- Main loop + interleaved tails ----
        pending_tail = []  # list of closures, 1 drained per jb iteration

        def emit_piece(pool, tag, csb, h, half, icc, dma_eng=None):
            tp = pool.tile([128, 65], F32, tag=tag)
            nc.tensor.transpose(
                tp, csb[:, icc * 128 : (icc + 1) * 128], ident[0:65, 0:65]
            )
            rz = rzp.tile([128, 1], F32, tag="rz")
            nc.vector.reciprocal(rz, tp[:, 64:65])
            ot = ob.tile([128, 64], F32, tag="ot")
            nc.vector.tensor_scalar_mul(ot, tp[:, 0:64], rz)
            (dma_eng or nc.sync).dma_start(
                out=out_r[half * 8 + icc, :, h * 64 : (h + 1) * 64], in_=ot
            )

        def make_tail(h, half, cp):
            csb = csbp.tile([65, 1024], F32, tag="csb")

            def piece_copy():
                nc.vector.tensor_copy(csb, cp)

            def make_piece(icc):
                eng = nc.gpsimd if icc % 2 else nc.sync
                return lambda: emit_piece(tpp, "tp", csb, h, half, icc, eng)

            last_tail_info.clear()
            last_tail_info.update(csb=csb, h=h, half=half)
            return [piece_copy] + [make_piece(i) for i in range(8)]

        last_tail_info = {}

        for h in range(4):
            p, sub = h // 2, h % 2
            qTh = q_T[sub * 64 : (sub + 1) * 64, p, :]
            for half in range(2):
                cp = cps.tile([65, 1024], F32, tag="ctx")

                def ctx_mms(jb, eb):
                    for nn in range(2):
                        nc.tensor.matmul(
                            cp[:, nn * 512 : (nn + 1) * 512],
                            q_nat[:, jb, h, 0:65],
                            eb[:, nn * 512 : (nn + 1) * 512],
                            start=(jb == 0),
                            stop=(jb == 15),
                        )

                pending = []
                for jb in range(16):  # key block (rows a)
                    sp = ps.tile([128, 1024], F32, tag="ps")
                    for nn in range(2):
                        nc.tensor.matmul(
                            sp[:, nn * 512 : (nn + 1) * 512],
                            qTh[:, jb * 128 : (jb + 1) * 128],
                            qTh[
                                :,
                                half * 1024 + nn * 512 : half * 1024 + (nn + 1) * 512,
                            ],
                            start=True,
                            stop=True,
                        )
                    eb = ep.tile([128, 1024], BF16, tag="eb")
                    nc.scalar.activation(
                        eb, sp, mybir.ActivationFunctionType.Exp, scale=0.125
                    )
                    pending.append((jb, eb))
                    # depth-2 pending: the first ctx (start=True) acquires
                    # the cps slot, which the previous block's copy holds
                    # for ~1.2us; emitting it 2 iterations in hides that.
                    while len(pending) > (1 if jb == 15 else 2):
                        ctx_mms(*pending.pop(0))
                    # drain one tail piece of the previous (h, half)
                    if pending_tail:
                        pending_tail.pop(0)()
                for pe_ in pending:
                    ctx_mms(*pe_)

                assert not pending_tail
                pending_tail = make_tail(h, half, cp)

        # Final tail: the ps pool is free now, so run the drain transposes
        # through its 2-slot rotation (2-deep pipelined against the DVE
        # normalize chain) instead of the single tpp bank, with a dummy
        # bridge covering the 1.2us accumulator copy.
        pending_tail[0]()
        for _ in range(8):
            dummy_mm()
        fi = last_tail_info
        csb, fh, fhalf = fi["csb"], fi["h"], fi["half"]
        for bb in range(2):
            tpb = ps.tile([128, 4, 65], F32, tag="ps")
            for j in range(4):
                icc = bb * 4 + j
                nc.tensor.transpose(
                    tpb[:, j, :], csb[:, icc * 128 : (icc + 1) * 128],
                    ident[0:65, 0:65],
                )
            rz4 = rzp.tile([128, 4], F32, tag="rz4")
            nc.vector.reciprocal(rz4, tpb[:, :, 64:65])
            for j in range(4):
                icc = bb * 4 + j
                ot = ob.tile([128, 64], F32, tag="ot")
                if j % 2:
                    nc.scalar.mul(ot, tpb[:, j, 0:64], rz4[:, j : j + 1])
                else:
                    nc.vector.tensor_scalar_mul(ot, tpb[:, j, 0:64], rz4[:, j : j + 1])
                eng = nc.gpsimd if icc % 2 else nc.sync
                eng.dma_start(
                    out=out_r[fhalf * 8 + icc, :, fh * 64 : (fh + 1) * 64], in_=ot
                )
            dummy_mm()
            dummy_mm()

    nc.compile()
    return nc


def get_program():
    global _PROGRAM
    if _PROGRAM is None:
        _PROGRAM = build_program()
    return _PROGRAM


def make_in_maps(x, Wq):
    x = np.asarray(x, dtype=np.float32)
    Wq = np.asarray(Wq, dtype=np.float32)
    in_maps = []
    for core in range(N_CORES):
        b, g = core // 2, core % 2
        # xt[p, ib, dc, i] = x[b][ib*128 + i, dc*128 + p]
        xt = np.ascontiguousarray(
            x[b].reshape(16, 128, 4, 128).transpose(3, 0, 2, 1).reshape(128, -1)
        )
        in_maps.append(
            {
                "xt": xt,
                "wq": np.ascontiguousarray(Wq[:, g * EC : (g + 1) * EC]),
            }
        )
    return in_maps


def assemble(results):
    out = np.empty((B, S, D), dtype=np.float32)
    for core in range(N_CORES):
        b, g = core // 2, core % 2
        out[b, :, g * EC : (g + 1) * EC] = results[core]["out"]
    return out


def kernel(x, Wq):
    from concourse.bass_utils import run_bass_kernel_spmd

    nc = get_program()
    res = run_bass_kernel_spmd(nc, make_in_maps(x, Wq), list(range(N_CORES)))
    return assemble(res.results)
